# revision 1
# baseline (speedup 1.0000x reference)
"""Causal self-attention (B=4, T=2048, C=1024, H=16, D=64) on 8 TRN2 cores.

Sharding: core c handles batch b = c//2 and head-half hh = c%2 (8 heads).
Each core computes qkv for its heads, attention, and a partial output
projection; the host sums the two partials per batch and adds b_proj.

Device kernel:
  - x transposed on PE (xT: [cin, t]) so qkv matmuls contract over cin.
  - q,k produced transposed per head-pair: qT/kT [128, T] bf16, partitions
    0:64 = head 2p, 64:128 = head 2p+1 (PE row-tiling runs the two K=64
    score matmuls concurrently).
  - scores as S^T [k, q] (k on partitions); att@v as out^T = v.T @ expS^T;
    v carries a ones column so the same accumulation produces the softmax
    denominator in output partition 64.
  - softmax normalization: DVE reciprocal + GPSIMD partition_broadcast +
    one DVE multiply per head.
  - fp32r (full-rate fp32) for qkv/proj matmuls; bf16 attention operands.
  - emission interleaves qkv chunk t4+1 into attention chunk qc=t4 at
    work-unit granularity so PE fills ACT-bound softmax stalls.
"""

import os

import numpy as np

import concourse.mybir as mybir
import concourse.tile as tile
from concourse import bacc
from concourse.bass_utils import run_bass_kernel_spmd

B, T, C = 4, 2048, 1024
H, D = 16, 64
HH = 512  # per-core head width: 8 heads * 64
N_CORES = 8

f32 = mybir.dt.float32
f32r = mybir.dt.float32r
bf16 = mybir.dt.bfloat16
EXP = mybir.ActivationFunctionType.Exp

_BUILT = None
LAST_RESULT = None  # BassKernelResults of the most recent run (for profiling)


def _interleave(a, b):
    """Merge unit lists: spread b evenly through a."""
    out = []
    na, nb = len(a), len(b)
    if na == 0:
        return list(b)
    bi = 0
    for i, u in enumerate(a):
        out.append(u)
        while bi < nb and (bi + 1) * na <= (i + 1) * nb:
            out.append(b[bi])
            bi += 1
    out.extend(b[bi:])
    return out


def _build():
    nc = bacc.Bacc("TRN2", target_bir_lowering=False, debug=False)

    x_d = nc.dram_tensor("xbT", [C, T], f32r, kind="ExternalInput")
    wq_d = nc.dram_tensor("wq", [C, HH], f32r, kind="ExternalInput")
    wk_d = nc.dram_tensor("wk", [C, HH], f32r, kind="ExternalInput")
    wv_d = nc.dram_tensor("wv", [C, HH], f32r, kind="ExternalInput")
    bq_d = nc.dram_tensor("bq", [HH], f32, kind="ExternalInput")
    bk_d = nc.dram_tensor("bk", [HH], f32, kind="ExternalInput")
    bv_d = nc.dram_tensor("bv", [HH], f32, kind="ExternalInput")
    wp_d = nc.dram_tensor("wp", [HH, C], f32r, kind="ExternalInput")
    y_d = nc.dram_tensor("y", [T, C], f32, kind="ExternalOutput")

    with tile.TileContext(nc) as tc:
        with (
            tc.tile_pool(name="persist", bufs=1) as P0,
            tc.tile_pool(name="psum", bufs=3, space="PSUM") as PS,
            tc.tile_pool(name="acc", bufs=1, space="PSUM") as PA,
            tc.tile_pool(name="wpool", bufs=1) as PW,
            tc.tile_pool(name="ph1", bufs=2) as P1,
            tc.tile_pool(name="ph2", bufs=2) as P2,
            tc.tile_pool(name="oTp", bufs=2) as P2o,
            tc.tile_pool(name="expp", bufs=4) as PEx,
        ):
            # Multiplicative causal masks for the 4 diagonal-crossing
            # positions: keep S^T[k, q] iff q - k - 128*d >= 0.
            masks = []
            for d in range(4):
                m = P0.tile([128, 512], bf16, tag=f"mask{d}", name=f"mask{d}")
                nc.gpsimd.memset(m[:, :], 1.0)
                nc.gpsimd.affine_select(
                    out=m[:, :],
                    in_=m[:, :],
                    compare_op=mybir.AluOpType.is_ge,
                    fill=0.0,
                    base=-128 * d,
                    pattern=[[1, 512]],
                    channel_multiplier=-1,
                )
                masks.append(m)

            # ones_row: row 0 = 1.0, rest 0 (bias injection via extra
            # contraction block in the v matmul)
            ones_stage = P0.tile([128, 128], f32, tag="ones_stage")
            nc.gpsimd.memset(ones_stage[:, :], 0.0)
            nc.gpsimd.memset(ones_stage[0:1, :], 1.0)
            ones_row = P0.tile([128, 128], f32r, tag="ones_row")
            nc.vector.tensor_copy(ones_row[:, :], ones_stage[:, :])

            bqk_sb = P0.tile([128, 8], f32, tag="bqk")
            for p in range(4):
                nc.sync.dma_start(
                    bqk_sb[:, p : p + 1], bq_d[128 * p : 128 * (p + 1), None]
                )
                nc.sync.dma_start(
                    bqk_sb[:, 4 + p : 5 + p], bk_d[128 * p : 128 * (p + 1), None]
                )
            bv_stage = P0.tile([128, 512], f32, tag="bv_stage")
            nc.gpsimd.memset(bv_stage[:, :], 0.0)
            nc.sync.dma_start(bv_stage[0:1, :], bv_d[None, :])
            bv_row = P0.tile([128, 512], f32r, tag="bv_row")
            nc.vector.tensor_copy(bv_row[:, :], bv_stage[:, :])

            wp_sb = P0.tile([128, 4, C], f32r, tag="wp")
            nc.sync.dma_start(
                wp_sb[:, :, :], wp_d[:, :].rearrange("(p u) c -> u p c", u=128)
            )

            qT = [
                P0.tile([128, T], bf16, tag=f"qT{p}", name=f"qT{p}")
                for p in range(4)
            ]
            kT = [
                P0.tile([128, T], bf16, tag=f"kT{p}", name=f"kT{p}")
                for p in range(4)
            ]
            # v with a ones column per head: [t, kb, head, 65]; column 64
            # is 1.0 so att@v also accumulates the softmax denominator.
            v_sb = P0.tile([128, 16, 8, 65], bf16, tag="v")
            nc.gpsimd.memset(v_sb[:, :, :, 64:65], 1.0)

            # Resident weights
            wvt = PW.tile([128, 8, HH], f32r, tag="wv")
            nc.sync.dma_start(
                wvt[:, :, :], wv_d[:, :].rearrange("(s u) m -> u s m", u=128)
            )
            wqt, wkt = [], []
            for p in range(4):
                wq_t = PW.tile([128, 8, 128], f32r, tag=f"wq{p}", name=f"wq{p}")
                nc.sync.dma_start(
                    wq_t[:, :, :],
                    wq_d[:, 128 * p : 128 * (p + 1)].rearrange(
                        "(s u) m -> u s m", u=128
                    ),
                )
                wqt.append(wq_t)
                wk_t = PW.tile([128, 8, 128], f32r, tag=f"wk{p}", name=f"wk{p}")
                nc.sync.dma_start(
                    wk_t[:, :, :],
                    wk_d[:, 128 * p : 128 * (p + 1)].rearrange(
                        "(s u) m -> u s m", u=128
                    ),
                )
                wkt.append(wk_t)

            # ---------- work-unit builders ----------

            def qkv_chunk_units(t4):
                """qkv for tokens [t4*512, (t4+1)*512): transposes, v, qT/kT."""
                units = []
                cell = {}

                def u_load(tbl, t4=t4, cell=cell):
                    if "xTc" not in cell:
                        cell["xTc"] = P1.tile(
                            [128, 8, 512], f32r, tag="xT", name="xTc"
                        )
                    xTc = cell["xTc"]
                    tb = 4 * t4 + tbl
                    nc.sync.dma_start(
                        xTc[:, :, tbl * 128 : (tbl + 1) * 128],
                        x_d[:, :].rearrange("(s u) t -> u s t", u=128)[
                            :, :, tb * 128 : (tb + 1) * 128
                        ],
                    )

                def u_v(tbl, t4=t4, cell=cell):
                    xTc = cell["xTc"]
                    tb = 4 * t4 + tbl
                    psv = PS.tile([128, 1024], f32, tag="s", name="psv")
                    for s in range(9):
                        lhsT = (
                            xTc[:, s, tbl * 128 : (tbl + 1) * 128]
                            if s < 8
                            else ones_row[:, :]
                        )
                        rhs = wvt[:, s, :] if s < 8 else bv_row[:, :]
                        nc.tensor.matmul(
                            psv[:, 0:512],
                            lhsT,
                            rhs,
                            start=(s == 0),
                            stop=(s == 8),
                        )
                    nc.vector.tensor_copy(
                        v_sb[:, tb, :, 0:64],
                        psv[:, 0:512].rearrange("p (h d) -> p h d", h=8),
                    )

                def u_qk(p, t4=t4, cell=cell):
                    xTc = cell["xTc"]
                    ps = PS.tile([128, 1024], f32, tag="s", name="psqk")
                    for s in range(8):
                        rhs = xTc[:, s, :]
                        nc.tensor.matmul(
                            ps[:, 0:512],
                            wqt[p][:, s, :],
                            rhs,
                            start=(s == 0),
                            stop=(s == 7),
                        )
                        nc.tensor.matmul(
                            ps[:, 512:1024],
                            wkt[p][:, s, :],
                            rhs,
                            start=(s == 0),
                            stop=(s == 7),
                        )
                    nc.vector.tensor_scalar_add(
                        qT[p][:, t4 * 512 : (t4 + 1) * 512],
                        ps[:, 0:512],
                        bqk_sb[:, p : p + 1],
                    )
                    nc.vector.tensor_scalar_add(
                        kT[p][:, t4 * 512 : (t4 + 1) * 512],
                        ps[:, 512:1024],
                        bqk_sb[:, 4 + p : 5 + p],
                    )

                for tbl in range(4):
                    units.append(lambda tbl=tbl: u_load(tbl))
                    units.append(lambda tbl=tbl: u_v(tbl))
                for p in range(4):
                    units.append(lambda p=p: u_qk(p))
                return units

            def att_chunk_units(qc):
                """Attention + projection for queries [qc*512, (qc+1)*512)."""
                units = []
                cell = {}
                kmax = 4 * qc + 4

                def u_pair_start(p, cell=cell):
                    cell["oA"] = PA.tile([128, 512], f32, tag="poA", name="poA")
                    cell["oB"] = PA.tile([128, 512], f32, tag="poB", name="poB")

                def u_kb(p, kb, qc=qc, cell=cell, kmax=kmax):
                    ps_s = PS.tile([128, 1024], f32, tag="s", name="ps_s")
                    ksl = slice(kb * 128, (kb + 1) * 128)
                    qsl = slice(qc * 512, (qc + 1) * 512)
                    nc.tensor.matmul(
                        ps_s[:, 0:512],
                        kT[p][0:64, ksl],
                        qT[p][0:64, qsl],
                        start=True,
                        stop=True,
                    )
                    nc.tensor.matmul(
                        ps_s[:, 512:1024],
                        kT[p][64:128, ksl],
                        qT[p][64:128, qsl],
                        start=True,
                        stop=True,
                    )
                    e2 = PEx.tile([128, 1024], bf16, tag="e", name="e2")
                    nc.scalar.activation(e2[:, :], ps_s[:, :], EXP, scale=0.125)
                    dg = kb - 4 * qc
                    if dg >= 0:
                        nc.vector.tensor_mul(
                            e2[:, 0:512], e2[:, 0:512], masks[dg][:, :]
                        )
                        nc.vector.tensor_mul(
                            e2[:, 512:1024], e2[:, 512:1024], masks[dg][:, :]
                        )
                    first, last = kb == 0, kb == kmax - 1
                    nc.tensor.matmul(
                        cell["oA"][0:65, :],
                        v_sb[:, kb, 2 * p, :],
                        e2[:, 0:512],
                        start=first,
                        stop=last,
                    )
                    nc.tensor.matmul(
                        cell["oB"][0:65, :],
                        v_sb[:, kb, 2 * p + 1, :],
                        e2[:, 512:1024],
                        start=first,
                        stop=last,
                    )

                def u_norm(p, cell=cell):
                    if "oT" not in cell:
                        cell["oT"] = P2o.tile(
                            [128, 4, 512], f32r, tag="oT", name="oT"
                        )
                    oT = cell["oT"]
                    rcA = P2.tile([1, 512], f32, tag="rcA", name="rcA")
                    rcB = P2.tile([1, 512], f32, tag="rcB", name="rcB")
                    nc.vector.reciprocal(rcA[:, :], cell["oA"][64:65, :])
                    nc.vector.reciprocal(rcB[:, :], cell["oB"][64:65, :])
                    bcA = P2.tile([64, 512], f32, tag="bcA", name="bcA")
                    bcB = P2.tile([64, 512], f32, tag="bcB", name="bcB")
                    nc.gpsimd.partition_broadcast(bcA[:, :], rcA[:, :])
                    nc.gpsimd.partition_broadcast(bcB[:, :], rcB[:, :])
                    nc.vector.tensor_mul(
                        oT[0:64, p, :], cell["oA"][0:64, :], bcA[:, :]
                    )
                    nc.vector.tensor_mul(
                        oT[64:128, p, :], cell["oB"][0:64, :], bcB[:, :]
                    )

                def u_proj(tb, cc, qc=qc, cell=cell):
                    oT = cell["oT"]
                    psy = PS.tile([128, 1024], f32, tag="s", name="psy")
                    for p in range(4):
                        nc.tensor.matmul(
                            psy[:, 0:512],
                            oT[:, p, tb * 128 : (tb + 1) * 128],
                            wp_sb[:, p, cc * 512 : (cc + 1) * 512],
                            start=(p == 0),
                            stop=(p == 3),
                        )
                    yst = P2.tile([128, 512], f32, tag="yst", name="yst")
                    nc.vector.tensor_copy(yst[:, :], psy[:, 0:512])
                    r0 = qc * 512 + tb * 128
                    nc.sync.dma_start(
                        y_d[r0 : r0 + 128, cc * 512 : (cc + 1) * 512],
                        yst[:, :],
                    )

                for p in range(4):
                    units.append(lambda p=p: u_pair_start(p))
                    for kb in range(kmax):
                        units.append(lambda p=p, kb=kb: u_kb(p, kb))
                    units.append(lambda p=p: u_norm(p))
                proj_units = [
                    (lambda tb=tb, cc=cc: u_proj(tb, cc))
                    for tb in range(4)
                    for cc in range(2)
                ]
                return units, proj_units

            # ---------- emission schedule ----------
            # qkv chunk 0 first; then attention(qc) with qkv chunk qc+1
            # spread through it so PE fills ACT-bound softmax stalls.
            for u in qkv_chunk_units(0):
                u()
            pending_proj = []
            for qc in range(4):
                att_units, proj_units = att_chunk_units(qc)
                nxt = pending_proj + (
                    qkv_chunk_units(qc + 1) if qc < 3 else []
                )
                for u in _interleave(att_units, nxt):
                    u()
                pending_proj = proj_units
            for u in pending_proj:
                u()

    nc.finalize()
    return nc


def _get_built():
    global _BUILT
    if _BUILT is None:
        _BUILT = _build()
    return _BUILT


def kernel(**inputs):
    global LAST_RESULT
    x = np.ascontiguousarray(np.asarray(inputs["x"], dtype=np.float32))
    w_qkv = np.ascontiguousarray(np.asarray(inputs["w_qkv"], dtype=np.float32))
    b_qkv = np.ascontiguousarray(np.asarray(inputs["b_qkv"], dtype=np.float32))
    w_proj = np.ascontiguousarray(np.asarray(inputs["w_proj"], dtype=np.float32))
    b_proj = np.ascontiguousarray(np.asarray(inputs["b_proj"], dtype=np.float32))

    nc = _get_built()
    in_maps = []
    for c in range(N_CORES):
        b, hh = c // 2, c % 2
        s = 512 * hh
        in_maps.append(
            {
                "xbT": np.ascontiguousarray(x[b].T),
                "wq": np.ascontiguousarray(w_qkv[:, s : s + 512]),
                "wk": np.ascontiguousarray(w_qkv[:, 1024 + s : 1024 + s + 512]),
                "wv": np.ascontiguousarray(w_qkv[:, 2048 + s : 2048 + s + 512]),
                "bq": np.ascontiguousarray(b_qkv[s : s + 512]),
                "bk": np.ascontiguousarray(b_qkv[1024 + s : 1024 + s + 512]),
                "bv": np.ascontiguousarray(b_qkv[2048 + s : 2048 + s + 512]),
                "wp": np.ascontiguousarray(w_proj[s : s + 512, :]),
            }
        )

    trace = bool(int(os.environ.get("KERNEL_TRACE", "0")))
    res = run_bass_kernel_spmd(
        nc, in_maps, core_ids=list(range(N_CORES)), trace=trace
    )
    LAST_RESULT = res
    out = np.empty((B, T, C), dtype=np.float32)
    for b in range(B):
        out[b] = (
            res.results[2 * b]["y"] + res.results[2 * b + 1]["y"] + b_proj[None, :]
        )
    return out



# revision 6
# speedup vs baseline: 1.3666x; 1.3666x over previous
"""Causal self-attention (B=4, T=2048, C=1024, H=16, D=64) on 8 TRN2 cores.

Sharding: core c handles batch b = c//2 and head-half hh = c%2 (8 heads).
Each core computes qkv for its heads, attention, and a partial output
projection; the host sums the two partials per batch and adds b_proj.

Device kernel:
  - All matmul operands bf16 (f32 PSUM accumulate): halves LDWEIGHTS time
    so weight loads hide under the previous matmul's 512-col stream, and
    halves input DMA (host casts x/w to bf16).
  - x transposed on host (xT: [cin, t]) so qkv matmuls contract over cin.
  - q,k produced transposed per head-pair: qT/kT [128, T] bf16, partitions
    0:64 = head 2p, 64:128 = head 2p+1 (PE row-tiling runs the two K=64
    score matmuls concurrently).
  - scores as S^T [k, q] (k on partitions); att@v as out^T = v.T @ expS^T;
    v carries a ones column so the same accumulation produces the softmax
    denominator in output partition 64.
  - softmax normalization: DVE reciprocal_approx_fast + Pool
    partition_broadcast + one DVE multiply per head.
  - scores(kb+1) emitted before att@v(kb) (depth-2 software pipeline) so
    the ACT exp latency is covered by PE work in program order.
  - qkv chunk t4+1 and projection work are interleaved into attention
    windows weighted by each window's ACT-vs-PE deficit; chunk 3's q/k
    for head-pairs 2,3 slide into window 3 to feed its tail.
"""

import os

import numpy as np

import concourse.mybir as mybir
import concourse.tile as tile
from concourse import bacc
from concourse.bass_utils import run_bass_kernel_spmd

B, T, C = 4, 2048, 1024
H, D = 16, 64
HH = 512  # per-core head width: 8 heads * 64
N_CORES = 8

f32 = mybir.dt.float32
bf16 = mybir.dt.bfloat16
EXP = mybir.ActivationFunctionType.Exp

_BUILT = None
LAST_RESULT = None  # BassKernelResults of the most recent run (for profiling)


def _interleave(a, b):
    """Merge unit lists: spread b evenly through a."""
    out = []
    na, nb = len(a), len(b)
    if na == 0:
        return list(b)
    bi = 0
    for i, u in enumerate(a):
        out.append(u)
        while bi < nb and (bi + 1) * na <= (i + 1) * nb:
            out.append(b[bi])
            bi += 1
    out.extend(b[bi:])
    return out


def _build():
    nc = bacc.Bacc("TRN2", target_bir_lowering=False, debug=False)

    x_d = nc.dram_tensor("xbT", [C, T], bf16, kind="ExternalInput")
    wq_d = nc.dram_tensor("wq", [C, HH], bf16, kind="ExternalInput")
    wk_d = nc.dram_tensor("wk", [C, HH], bf16, kind="ExternalInput")
    wv_d = nc.dram_tensor("wv", [C, HH], bf16, kind="ExternalInput")
    bq_d = nc.dram_tensor("bq", [HH], f32, kind="ExternalInput")
    bk_d = nc.dram_tensor("bk", [HH], f32, kind="ExternalInput")
    bv_d = nc.dram_tensor("bv", [HH], f32, kind="ExternalInput")
    wp_d = nc.dram_tensor("wp", [HH, C], bf16, kind="ExternalInput")
    y_d = nc.dram_tensor("y", [T, C], f32, kind="ExternalOutput")

    with tile.TileContext(nc) as tc:
        with (
            tc.tile_pool(name="persist", bufs=1) as P0,
            tc.tile_pool(name="psum", bufs=3, space="PSUM") as PS,
            tc.tile_pool(name="acc", bufs=1, space="PSUM") as PA,
            tc.tile_pool(name="wpool", bufs=1) as PW,
            tc.tile_pool(name="ph1", bufs=2) as P1,
            tc.tile_pool(name="ph2", bufs=2) as P2,
            tc.tile_pool(name="oTp", bufs=3) as P2o,
            tc.tile_pool(name="expp", bufs=4) as PEx,
        ):
            # Resident weights first so their DMA queues ahead of masks.
            wvt = PW.tile([128, 8, HH], bf16, tag="wv")
            nc.sync.dma_start(
                wvt[:, :, :], wv_d[:, :].rearrange("(s u) m -> u s m", u=128)
            )
            wqt, wkt = [], []
            for p in range(4):
                wq_t = PW.tile([128, 8, 128], bf16, tag=f"wq{p}", name=f"wq{p}")
                nc.sync.dma_start(
                    wq_t[:, :, :],
                    wq_d[:, 128 * p : 128 * (p + 1)].rearrange(
                        "(s u) m -> u s m", u=128
                    ),
                )
                wqt.append(wq_t)
                wk_t = PW.tile([128, 8, 128], bf16, tag=f"wk{p}", name=f"wk{p}")
                nc.sync.dma_start(
                    wk_t[:, :, :],
                    wk_d[:, 128 * p : 128 * (p + 1)].rearrange(
                        "(s u) m -> u s m", u=128
                    ),
                )
                wkt.append(wk_t)

            # Multiplicative causal masks for the 4 diagonal-crossing
            # positions: keep S^T[k, q] iff q - k - 128*d >= 0.
            masks = []
            for d in range(4):
                m = P0.tile([128, 512], bf16, tag=f"mask{d}", name=f"mask{d}")
                nc.gpsimd.memset(m[:, :], 1.0)
                nc.gpsimd.affine_select(
                    out=m[:, :],
                    in_=m[:, :],
                    compare_op=mybir.AluOpType.is_ge,
                    fill=0.0,
                    base=-128 * d,
                    pattern=[[1, 512]],
                    channel_multiplier=-1,
                )
                masks.append(m)

            # ones_row: row 0 = 1.0, rest 0 (bias injection via extra
            # contraction block in the v matmul)
            ones_stage = P0.tile([128, 128], f32, tag="ones_stage")
            nc.gpsimd.memset(ones_stage[:, :], 0.0)
            nc.gpsimd.memset(ones_stage[0:1, :], 1.0)
            ones_row = P0.tile([128, 128], bf16, tag="ones_row")
            nc.vector.tensor_copy(ones_row[:, :], ones_stage[:, :])

            bqk_sb = P0.tile([128, 8], f32, tag="bqk")
            for p in range(4):
                nc.sync.dma_start(
                    bqk_sb[:, p : p + 1], bq_d[128 * p : 128 * (p + 1), None]
                )
                nc.sync.dma_start(
                    bqk_sb[:, 4 + p : 5 + p], bk_d[128 * p : 128 * (p + 1), None]
                )
            bv_stage = P0.tile([128, 512], f32, tag="bv_stage")
            nc.gpsimd.memset(bv_stage[:, :], 0.0)
            nc.sync.dma_start(bv_stage[0:1, :], bv_d[None, :])
            bv_row = P0.tile([128, 512], bf16, tag="bv_row")
            nc.vector.tensor_copy(bv_row[:, :], bv_stage[:, :])

            qT = [
                P0.tile([128, T], bf16, tag=f"qT{p}", name=f"qT{p}")
                for p in range(4)
            ]
            kT = [
                P0.tile([128, T], bf16, tag=f"kT{p}", name=f"kT{p}")
                for p in range(4)
            ]
            # v with a ones column per head: [t, kb, head, 65]; column 64
            # is 1.0 so att@v also accumulates the softmax denominator.
            v_sb = P0.tile([128, 16, 8, 65], bf16, tag="v")
            nc.gpsimd.memset(v_sb[:, :, :, 64:65], 1.0)

            # wp loaded lazily (first needed ~100us in, after window 0)
            wp_sb = P0.tile([128, 4, C], bf16, tag="wp")

            def u_load_wp():
                nc.sync.dma_start(
                    wp_sb[:, :, :],
                    wp_d[:, :].rearrange("(p u) c -> u p c", u=128),
                )

            # ---------- work-unit builders ----------

            def qkv_chunk_units(t4):
                """qkv for tokens [t4*512, (t4+1)*512): transposes, v, qT/kT.

                Returns (pre, qk) unit lists; qk[p] may be deferred into a
                later window than pre (loads + v), but every unit of pre
                must be emitted before any unit of qk.
                """
                pre, qk = [], []
                cell = {}

                def u_load(tbl, t4=t4, cell=cell):
                    if "xTc" not in cell:
                        cell["xTc"] = P1.tile(
                            [128, 8, 512], bf16, tag="xT", name="xTc"
                        )
                    xTc = cell["xTc"]
                    tb = 4 * t4 + tbl
                    nc.sync.dma_start(
                        xTc[:, :, tbl * 128 : (tbl + 1) * 128],
                        x_d[:, :].rearrange("(s u) t -> u s t", u=128)[
                            :, :, tb * 128 : (tb + 1) * 128
                        ],
                    )

                def u_v(tbl, t4=t4, cell=cell):
                    xTc = cell["xTc"]
                    tb = 4 * t4 + tbl
                    psv = PS.tile([128, 1024], f32, tag="s", name="psv")
                    for s in range(9):
                        lhsT = (
                            xTc[:, s, tbl * 128 : (tbl + 1) * 128]
                            if s < 8
                            else ones_row[:, :]
                        )
                        rhs = wvt[:, s, :] if s < 8 else bv_row[:, :]
                        nc.tensor.matmul(
                            psv[:, 0:512],
                            lhsT,
                            rhs,
                            start=(s == 0),
                            stop=(s == 8),
                        )
                    nc.vector.tensor_copy(
                        v_sb[:, tb, :, 0:64],
                        psv[:, 0:512].rearrange("p (h d) -> p h d", h=8),
                    )

                def u_qk(p, t4=t4, cell=cell):
                    xTc = cell["xTc"]
                    ps = PS.tile([128, 1024], f32, tag="s", name="psqk")
                    for s in range(8):
                        rhs = xTc[:, s, :]
                        nc.tensor.matmul(
                            ps[:, 0:512],
                            wqt[p][:, s, :],
                            rhs,
                            start=(s == 0),
                            stop=(s == 7),
                        )
                        nc.tensor.matmul(
                            ps[:, 512:1024],
                            wkt[p][:, s, :],
                            rhs,
                            start=(s == 0),
                            stop=(s == 7),
                        )
                    nc.vector.tensor_scalar_add(
                        qT[p][:, t4 * 512 : (t4 + 1) * 512],
                        ps[:, 0:512],
                        bqk_sb[:, p : p + 1],
                    )
                    nc.vector.tensor_scalar_add(
                        kT[p][:, t4 * 512 : (t4 + 1) * 512],
                        ps[:, 512:1024],
                        bqk_sb[:, 4 + p : 5 + p],
                    )

                for tbl in range(4):
                    pre.append(lambda tbl=tbl: u_load(tbl))
                    pre.append(lambda tbl=tbl: u_v(tbl))
                for p in range(4):
                    qk.append(lambda p=p: u_qk(p))
                return pre, qk

            def att_chunk_units(qc):
                """Attention + projection for queries [qc*512, (qc+1)*512).

                Returns (sections_ab, sections_cd, proj_units): head-pair
                sections p=0,1 and p=2,3 as separate unit lists (so fillers
                that must precede p>=2 can interleave into the first half).
                Each u_scores(kb) is emitted one step ahead of u_av(kb-1):
                the ACT exp of kb-1 overlaps scores kb + fillers on PE.
                """
                kmax = 4 * qc + 4
                cell = {}

                def u_pair_start(p, cell=cell):
                    cell["oA"] = PA.tile([128, 512], f32, tag="poA", name="poA")
                    cell["oB"] = PA.tile([128, 512], f32, tag="poB", name="poB")

                def u_scores(p, kb, qc=qc, cell=cell):
                    ps_s = PS.tile([128, 1024], f32, tag="s", name="ps_s")
                    ksl = slice(kb * 128, (kb + 1) * 128)
                    qsl = slice(qc * 512, (qc + 1) * 512)
                    nc.tensor.matmul(
                        ps_s[:, 0:512],
                        kT[p][0:64, ksl],
                        qT[p][0:64, qsl],
                        start=True,
                        stop=True,
                    )
                    nc.tensor.matmul(
                        ps_s[:, 512:1024],
                        kT[p][64:128, ksl],
                        qT[p][64:128, qsl],
                        start=True,
                        stop=True,
                    )
                    e2 = PEx.tile([128, 1024], bf16, tag="e", name="e2")
                    nc.scalar.activation(e2[:, :], ps_s[:, :], EXP, scale=0.125)
                    dg = kb - 4 * qc
                    if dg >= 0:
                        nc.vector.tensor_mul(
                            e2[:, 0:512], e2[:, 0:512], masks[dg][:, :]
                        )
                        nc.vector.tensor_mul(
                            e2[:, 512:1024], e2[:, 512:1024], masks[dg][:, :]
                        )
                    cell[("e", kb)] = e2

                def u_av(p, kb, cell=cell, kmax=kmax):
                    e2 = cell.pop(("e", kb))
                    first, last = kb == 0, kb == kmax - 1
                    nc.tensor.matmul(
                        cell["oA"][0:65, :],
                        v_sb[:, kb, 2 * p, :],
                        e2[:, 0:512],
                        start=first,
                        stop=last,
                    )
                    nc.tensor.matmul(
                        cell["oB"][0:65, :],
                        v_sb[:, kb, 2 * p + 1, :],
                        e2[:, 512:1024],
                        start=first,
                        stop=last,
                    )

                def u_norm(p, cell=cell):
                    if "oT" not in cell:
                        cell["oT"] = P2o.tile(
                            [128, 4, 512], bf16, tag="oT", name="oT"
                        )
                    oT = cell["oT"]
                    # recip_approx_fast misreads partition-offset PSUM APs:
                    # stage both denominator rows to partition 0 first.
                    dst = P2.tile([1, 1024], f32, tag="rst", name="rst")
                    nc.vector.tensor_copy(dst[:, 0:512], cell["oA"][64:65, :])
                    nc.vector.tensor_copy(dst[:, 512:1024], cell["oB"][64:65, :])
                    rc = P2.tile([1, 1024], f32, tag="rc", name="rc")
                    nc.vector.reciprocal_approx_fast(out=rc[:, :], in_=dst[:, :])
                    bcA = P2.tile([64, 512], f32, tag="bcA", name="bcA")
                    bcB = P2.tile([64, 512], f32, tag="bcB", name="bcB")
                    nc.gpsimd.partition_broadcast(bcA[:, :], rc[:, 0:512])
                    nc.gpsimd.partition_broadcast(bcB[:, :], rc[:, 512:1024])
                    nc.vector.tensor_mul(
                        oT[0:64, p, :], cell["oA"][0:64, :], bcA[:, :]
                    )
                    nc.vector.tensor_mul(
                        oT[64:128, p, :], cell["oB"][0:64, :], bcB[:, :]
                    )

                def u_proj(tb, cc, qc=qc, cell=cell):
                    oT = cell["oT"]
                    psy = PS.tile([128, 1024], f32, tag="s", name="psy")
                    for p in range(4):
                        nc.tensor.matmul(
                            psy[:, 0:512],
                            oT[:, p, tb * 128 : (tb + 1) * 128],
                            wp_sb[:, p, cc * 512 : (cc + 1) * 512],
                            start=(p == 0),
                            stop=(p == 3),
                        )
                    yst = P2.tile([128, 512], f32, tag="yst", name="yst")
                    nc.vector.tensor_copy(yst[:, :], psy[:, 0:512])
                    r0 = qc * 512 + tb * 128
                    nc.sync.dma_start(
                        y_d[r0 : r0 + 128, cc * 512 : (cc + 1) * 512],
                        yst[:, :],
                    )

                def section(p):
                    units = [lambda p=p: u_pair_start(p)]
                    units.append(lambda p=p: u_scores(p, 0))
                    for kb in range(1, kmax):
                        units.append(lambda p=p, kb=kb: u_scores(p, kb))
                        units.append(lambda p=p, kb=kb: u_av(p, kb - 1))
                    units.append(lambda p=p: u_av(p, kmax - 1))
                    units.append(lambda p=p: u_norm(p))
                    return units

                sections_ab = section(0) + section(1)
                sections_cd = section(2) + section(3)
                proj_units = [
                    (lambda tb=tb, cc=cc: u_proj(tb, cc))
                    for tb in range(4)
                    for cc in range(2)
                ]
                return sections_ab, sections_cd, proj_units

            # ---------- emission schedule ----------
            # chunk0 first; windows qc=0..3 with fillers weighted by each
            # window's ACT-vs-PE deficit. chunk3's qk for p=2,3 slide into
            # window 3 (they are only needed by its p=2,3 sections).
            pre0, qk0 = qkv_chunk_units(0)
            for u in pre0 + qk0:
                u()

            pre1, qk1 = qkv_chunk_units(1)
            pre2, qk2 = qkv_chunk_units(2)
            pre3, qk3 = qkv_chunk_units(3)

            ab0, cd0, proj0 = att_chunk_units(0)
            for u in _interleave(ab0 + cd0, pre1 + qk1):
                u()

            ab1, cd1, proj1 = att_chunk_units(1)
            fill1 = pre2 + qk2 + [u_load_wp] + proj0[0:4]
            for u in _interleave(ab1 + cd1, fill1):
                u()

            ab2, cd2, proj2 = att_chunk_units(2)
            fill2 = pre3 + qk3[0:2] + proj0[4:8] + proj1[0:2]
            for u in _interleave(ab2 + cd2, fill2):
                u()

            ab3, cd3, proj3 = att_chunk_units(3)
            for u in _interleave(ab3, qk3[2:4] + proj1[2:8]):
                u()
            for u in _interleave(cd3, proj2):
                u()
            for u in proj3:
                u()

    nc.finalize()
    return nc


def _get_built():
    global _BUILT
    if _BUILT is None:
        _BUILT = _build()
    return _BUILT


def kernel(**inputs):
    global LAST_RESULT
    import ml_dtypes

    bfloat16 = ml_dtypes.bfloat16
    x = np.asarray(inputs["x"], dtype=np.float32)
    w_qkv = np.asarray(inputs["w_qkv"], dtype=np.float32)
    b_qkv = np.ascontiguousarray(np.asarray(inputs["b_qkv"], dtype=np.float32))
    w_proj = np.asarray(inputs["w_proj"], dtype=np.float32)
    b_proj = np.ascontiguousarray(np.asarray(inputs["b_proj"], dtype=np.float32))

    nc = _get_built()
    in_maps = []
    for c in range(N_CORES):
        b, hh = c // 2, c % 2
        s = 512 * hh
        in_maps.append(
            {
                "xbT": np.ascontiguousarray(x[b].T.astype(bfloat16)),
                "wq": np.ascontiguousarray(
                    w_qkv[:, s : s + 512].astype(bfloat16)
                ),
                "wk": np.ascontiguousarray(
                    w_qkv[:, 1024 + s : 1024 + s + 512].astype(bfloat16)
                ),
                "wv": np.ascontiguousarray(
                    w_qkv[:, 2048 + s : 2048 + s + 512].astype(bfloat16)
                ),
                "bq": np.ascontiguousarray(b_qkv[s : s + 512]),
                "bk": np.ascontiguousarray(b_qkv[1024 + s : 1024 + s + 512]),
                "bv": np.ascontiguousarray(b_qkv[2048 + s : 2048 + s + 512]),
                "wp": np.ascontiguousarray(
                    w_proj[s : s + 512, :].astype(bfloat16)
                ),
            }
        )

    trace = bool(int(os.environ.get("KERNEL_TRACE", "0")))
    res = run_bass_kernel_spmd(
        nc, in_maps, core_ids=list(range(N_CORES)), trace=trace
    )
    LAST_RESULT = res
    out = np.empty((B, T, C), dtype=np.float32)
    for b in range(B):
        out[b] = (
            res.results[2 * b]["y"] + res.results[2 * b + 1]["y"] + b_proj[None, :]
        )
    return out


# revision 7
# speedup vs baseline: 1.4067x; 1.0294x over previous
"""Causal self-attention (B=4, T=2048, C=1024, H=16, D=64) on 8 TRN2 cores.

Sharding: core c handles batch b = c//2 and head-half hh = c%2 (8 heads).
Each core computes qkv for its heads, attention, and a partial output
projection; the host sums the two partials per batch and adds b_proj.

Device kernel:
  - All matmul operands bf16 (f32 PSUM accumulate): halves LDWEIGHTS time
    so weight loads hide under the previous matmul's 512-col stream, and
    halves input DMA (host casts x/w to bf16).
  - x transposed on host (xT: [cin, t]) so qkv matmuls contract over cin.
  - q,k produced transposed per head-pair: qT/kT [128, T] bf16, partitions
    0:64 = head 2p, 64:128 = head 2p+1 (PE row-tiling runs the two K=64
    score matmuls concurrently).
  - scores as S^T [k, q] (k on partitions); att@v as out^T = v.T @ expS^T;
    v carries a ones column so the same accumulation produces the softmax
    denominator in output partition 64.
  - softmax normalization: DVE reciprocal_approx_fast + Pool
    partition_broadcast + one DVE multiply per head.
  - scores(kb+1) emitted before att@v(kb) (depth-2 software pipeline) so
    the ACT exp latency is covered by PE work in program order.
  - qkv chunk t4+1 and projection work are interleaved into attention
    windows weighted by each window's ACT-vs-PE deficit; chunk 3's q/k
    for head-pairs 2,3 slide into window 3 to feed its tail.
"""

import os

import numpy as np

import concourse.mybir as mybir
import concourse.tile as tile
from concourse import bacc
from concourse.bass_utils import run_bass_kernel_spmd

B, T, C = 4, 2048, 1024
H, D = 16, 64
HH = 512  # per-core head width: 8 heads * 64
N_CORES = 8

f32 = mybir.dt.float32
bf16 = mybir.dt.bfloat16
EXP = mybir.ActivationFunctionType.Exp

_BUILT = None
LAST_RESULT = None  # BassKernelResults of the most recent run (for profiling)


def _interleave(a, b):
    """Merge unit lists: spread b evenly through a."""
    out = []
    na, nb = len(a), len(b)
    if na == 0:
        return list(b)
    bi = 0
    for i, u in enumerate(a):
        out.append(u)
        while bi < nb and (bi + 1) * na <= (i + 1) * nb:
            out.append(b[bi])
            bi += 1
    out.extend(b[bi:])
    return out


def _build():
    nc = bacc.Bacc("TRN2", target_bir_lowering=False, debug=False)

    x_d = nc.dram_tensor("xbT", [C, T], bf16, kind="ExternalInput")
    wq_d = nc.dram_tensor("wq", [C, HH], bf16, kind="ExternalInput")
    wk_d = nc.dram_tensor("wk", [C, HH], bf16, kind="ExternalInput")
    wv_d = nc.dram_tensor("wv", [C, HH], bf16, kind="ExternalInput")
    bq_d = nc.dram_tensor("bq", [HH], f32, kind="ExternalInput")
    bk_d = nc.dram_tensor("bk", [HH], f32, kind="ExternalInput")
    bv_d = nc.dram_tensor("bv", [HH], f32, kind="ExternalInput")
    wp_d = nc.dram_tensor("wp", [HH, C], bf16, kind="ExternalInput")
    y_d = nc.dram_tensor("y", [T, C], f32, kind="ExternalOutput")

    with tile.TileContext(nc) as tc:
        with (
            tc.tile_pool(name="persist", bufs=1) as P0,
            tc.tile_pool(name="psum", bufs=3, space="PSUM") as PS,
            tc.tile_pool(name="acc", bufs=1, space="PSUM") as PA,
            tc.tile_pool(name="wpool", bufs=1) as PW,
            tc.tile_pool(name="ph1", bufs=2) as P1,
            tc.tile_pool(name="ph2", bufs=2) as P2,
            tc.tile_pool(name="oTp", bufs=3) as P2o,
            tc.tile_pool(name="expp", bufs=4) as PEx,
        ):
            # Resident weights first so their DMA queues ahead of masks.
            wvt = PW.tile([128, 8, HH], bf16, tag="wv")
            nc.sync.dma_start(
                wvt[:, :, :], wv_d[:, :].rearrange("(s u) m -> u s m", u=128)
            )
            wqt, wkt = [], []
            for p in range(4):
                wq_t = PW.tile([128, 8, 128], bf16, tag=f"wq{p}", name=f"wq{p}")
                nc.sync.dma_start(
                    wq_t[:, :, :],
                    wq_d[:, 128 * p : 128 * (p + 1)].rearrange(
                        "(s u) m -> u s m", u=128
                    ),
                )
                wqt.append(wq_t)
                wk_t = PW.tile([128, 8, 128], bf16, tag=f"wk{p}", name=f"wk{p}")
                nc.sync.dma_start(
                    wk_t[:, :, :],
                    wk_d[:, 128 * p : 128 * (p + 1)].rearrange(
                        "(s u) m -> u s m", u=128
                    ),
                )
                wkt.append(wk_t)

            # Multiplicative causal masks for the 4 diagonal-crossing
            # positions: keep S^T[k, q] iff q - k - 128*d >= 0.
            masks = []
            for d in range(4):
                m = P0.tile([128, 1024], bf16, tag=f"mask{d}", name=f"mask{d}")
                nc.gpsimd.memset(m[:, :], 1.0)
                mr = m[:, :].rearrange("p (h q) -> p h q", h=2)
                nc.gpsimd.affine_select(
                    out=mr,
                    in_=mr,
                    compare_op=mybir.AluOpType.is_ge,
                    fill=0.0,
                    base=-128 * d,
                    pattern=[[0, 2], [1, 512]],
                    channel_multiplier=-1,
                )
                masks.append(m)

            # ones_row: row 0 = 1.0, rest 0 (bias injection via extra
            # contraction block in the v matmul)
            ones_stage = P0.tile([128, 128], f32, tag="ones_stage")
            nc.gpsimd.memset(ones_stage[:, :], 0.0)
            nc.gpsimd.memset(ones_stage[0:1, :], 1.0)
            ones_row = P0.tile([128, 128], bf16, tag="ones_row")
            nc.vector.tensor_copy(ones_row[:, :], ones_stage[:, :])

            bqk_sb = P0.tile([128, 8], f32, tag="bqk")
            for p in range(4):
                nc.sync.dma_start(
                    bqk_sb[:, p : p + 1], bq_d[128 * p : 128 * (p + 1), None]
                )
                nc.sync.dma_start(
                    bqk_sb[:, 4 + p : 5 + p], bk_d[128 * p : 128 * (p + 1), None]
                )
            bv_stage = P0.tile([128, 512], f32, tag="bv_stage")
            nc.gpsimd.memset(bv_stage[:, :], 0.0)
            nc.sync.dma_start(bv_stage[0:1, :], bv_d[None, :])
            bv_row = P0.tile([128, 512], bf16, tag="bv_row")
            nc.vector.tensor_copy(bv_row[:, :], bv_stage[:, :])

            qT = [
                P0.tile([128, T], bf16, tag=f"qT{p}", name=f"qT{p}")
                for p in range(4)
            ]
            kT = [
                P0.tile([128, T], bf16, tag=f"kT{p}", name=f"kT{p}")
                for p in range(4)
            ]
            # v with a ones column per head: [t, kb, head, 65]; column 64
            # is 1.0 so att@v also accumulates the softmax denominator.
            v_sb = P0.tile([128, 16, 8, 65], bf16, tag="v")
            nc.gpsimd.memset(v_sb[:, :, :, 64:65], 1.0)

            # wp loaded lazily (first needed ~100us in, after window 0)
            wp_sb = P0.tile([128, 4, C], bf16, tag="wp")

            def u_load_wp():
                nc.sync.dma_start(
                    wp_sb[:, :, :],
                    wp_d[:, :].rearrange("(p u) c -> u p c", u=128),
                )

            # ---------- work-unit builders ----------

            def qkv_chunk_units(t4):
                """qkv for tokens [t4*512, (t4+1)*512): transposes, v, qT/kT.

                Returns (pre, qk) unit lists; qk[p] may be deferred into a
                later window than pre (loads + v), but every unit of pre
                must be emitted before any unit of qk.
                """
                pre, qk = [], []
                cell = {}

                def u_load(tbl, t4=t4, cell=cell):
                    if "xTc" not in cell:
                        cell["xTc"] = P1.tile(
                            [128, 8, 512], bf16, tag="xT", name="xTc"
                        )
                    xTc = cell["xTc"]
                    tb = 4 * t4 + tbl
                    nc.sync.dma_start(
                        xTc[:, :, tbl * 128 : (tbl + 1) * 128],
                        x_d[:, :].rearrange("(s u) t -> u s t", u=128)[
                            :, :, tb * 128 : (tb + 1) * 128
                        ],
                    )

                def u_v(tbl, t4=t4, cell=cell):
                    xTc = cell["xTc"]
                    tb = 4 * t4 + tbl
                    psv = PS.tile([128, 1024], f32, tag="s", name="psv")
                    for s in range(9):
                        lhsT = (
                            xTc[:, s, tbl * 128 : (tbl + 1) * 128]
                            if s < 8
                            else ones_row[:, :]
                        )
                        rhs = wvt[:, s, :] if s < 8 else bv_row[:, :]
                        nc.tensor.matmul(
                            psv[:, 0:512],
                            lhsT,
                            rhs,
                            start=(s == 0),
                            stop=(s == 8),
                        )
                    nc.vector.tensor_copy(
                        v_sb[:, tb, :, 0:64],
                        psv[:, 0:512].rearrange("p (h d) -> p h d", h=8),
                    )

                def u_qk(p, t4=t4, cell=cell):
                    xTc = cell["xTc"]
                    ps = PS.tile([128, 1024], f32, tag="s", name="psqk")
                    for s in range(8):
                        rhs = xTc[:, s, :]
                        nc.tensor.matmul(
                            ps[:, 0:512],
                            wqt[p][:, s, :],
                            rhs,
                            start=(s == 0),
                            stop=(s == 7),
                        )
                        nc.tensor.matmul(
                            ps[:, 512:1024],
                            wkt[p][:, s, :],
                            rhs,
                            start=(s == 0),
                            stop=(s == 7),
                        )
                    nc.vector.tensor_scalar_add(
                        qT[p][:, t4 * 512 : (t4 + 1) * 512],
                        ps[:, 0:512],
                        bqk_sb[:, p : p + 1],
                    )
                    nc.vector.tensor_scalar_add(
                        kT[p][:, t4 * 512 : (t4 + 1) * 512],
                        ps[:, 512:1024],
                        bqk_sb[:, 4 + p : 5 + p],
                    )

                for tbl in range(4):
                    pre.append(lambda tbl=tbl: u_load(tbl))
                    pre.append(lambda tbl=tbl: u_v(tbl))
                for p in range(4):
                    qk.append(lambda p=p: u_qk(p))
                return pre, qk

            def att_chunk_units(qc):
                """Attention + projection for queries [qc*512, (qc+1)*512).

                Returns (sections_ab, sections_cd, proj_units): head-pair
                sections p=0,1 and p=2,3 as separate unit lists (so fillers
                that must precede p>=2 can interleave into the first half).
                Each u_scores(kb) is emitted one step ahead of u_av(kb-1):
                the ACT exp of kb-1 overlaps scores kb + fillers on PE.
                """
                kmax = 4 * qc + 4
                cell = {}

                def u_pair_start(p, cell=cell):
                    cell["oA"] = PA.tile([128, 512], f32, tag="poA", name="poA")
                    cell["oB"] = PA.tile([128, 512], f32, tag="poB", name="poB")

                def u_scores(p, kb, qc=qc, cell=cell):
                    # Diagonal-crossing blocks only have valid scores for
                    # q >= off = 128*dg: restrict matmul/exp/mask to that
                    # q-range (cols below off stay stale and unread).
                    dg = kb - 4 * qc
                    off = 128 * dg if dg > 0 else 0
                    ps_s = PS.tile([128, 1024], f32, tag="s", name="ps_s")
                    ksl = slice(kb * 128, (kb + 1) * 128)
                    qsl = slice(qc * 512 + off, (qc + 1) * 512)
                    nc.tensor.matmul(
                        ps_s[:, off:512],
                        kT[p][0:64, ksl],
                        qT[p][0:64, qsl],
                        start=True,
                        stop=True,
                    )
                    nc.tensor.matmul(
                        ps_s[:, 512 + off : 1024],
                        kT[p][64:128, ksl],
                        qT[p][64:128, qsl],
                        start=True,
                        stop=True,
                    )
                    e2 = PEx.tile([128, 1024], bf16, tag="e", name="e2")
                    if off:
                        e2r = e2[:, :].rearrange("p (h q) -> p h q", h=2)[
                            :, :, off:512
                        ]
                        psr = ps_s[:, :].rearrange("p (h q) -> p h q", h=2)[
                            :, :, off:512
                        ]
                        mr = masks[dg][:, :].rearrange(
                            "p (h q) -> p h q", h=2
                        )[:, :, off:512]
                        nc.scalar.activation(e2r, psr, EXP, scale=0.125)
                        nc.vector.tensor_mul(e2r, e2r, mr)
                    else:
                        nc.scalar.activation(
                            e2[:, :], ps_s[:, :], EXP, scale=0.125
                        )
                        if dg == 0:
                            nc.vector.tensor_mul(
                                e2[:, :], e2[:, :], masks[0][:, :]
                            )
                    cell[("e", kb)] = (e2, off)

                def u_av(p, kb, cell=cell, kmax=kmax):
                    e2, off = cell.pop(("e", kb))
                    first, last = kb == 0, kb == kmax - 1
                    nc.tensor.matmul(
                        cell["oA"][0:65, off:512],
                        v_sb[:, kb, 2 * p, :],
                        e2[:, off:512],
                        start=first,
                        stop=last,
                    )
                    nc.tensor.matmul(
                        cell["oB"][0:65, off:512],
                        v_sb[:, kb, 2 * p + 1, :],
                        e2[:, 512 + off : 1024],
                        start=first,
                        stop=last,
                    )

                def u_norm(p, cell=cell):
                    if "oT" not in cell:
                        cell["oT"] = P2o.tile(
                            [128, 4, 512], bf16, tag="oT", name="oT"
                        )
                    oT = cell["oT"]
                    # recip_approx_fast misreads partition-offset PSUM APs:
                    # stage both denominator rows to partition 0 first.
                    dst = P2.tile([1, 1024], f32, tag="rst", name="rst")
                    nc.vector.tensor_copy(dst[:, 0:512], cell["oA"][64:65, :])
                    nc.vector.tensor_copy(dst[:, 512:1024], cell["oB"][64:65, :])
                    rc = P2.tile([1, 1024], f32, tag="rc", name="rc")
                    nc.vector.reciprocal_approx_fast(out=rc[:, :], in_=dst[:, :])
                    bcA = P2.tile([64, 512], f32, tag="bcA", name="bcA")
                    bcB = P2.tile([64, 512], f32, tag="bcB", name="bcB")
                    nc.gpsimd.partition_broadcast(bcA[:, :], rc[:, 0:512])
                    nc.gpsimd.partition_broadcast(bcB[:, :], rc[:, 512:1024])
                    nc.vector.tensor_mul(
                        oT[0:64, p, :], cell["oA"][0:64, :], bcA[:, :]
                    )
                    nc.vector.tensor_mul(
                        oT[64:128, p, :], cell["oB"][0:64, :], bcB[:, :]
                    )

                def u_proj(tb, cc, qc=qc, cell=cell):
                    oT = cell["oT"]
                    psy = PS.tile([128, 1024], f32, tag="s", name="psy")
                    for p in range(4):
                        nc.tensor.matmul(
                            psy[:, 0:512],
                            oT[:, p, tb * 128 : (tb + 1) * 128],
                            wp_sb[:, p, cc * 512 : (cc + 1) * 512],
                            start=(p == 0),
                            stop=(p == 3),
                        )
                    yst = P2.tile([128, 512], f32, tag="yst", name="yst")
                    nc.vector.tensor_copy(yst[:, :], psy[:, 0:512])
                    r0 = qc * 512 + tb * 128
                    nc.sync.dma_start(
                        y_d[r0 : r0 + 128, cc * 512 : (cc + 1) * 512],
                        yst[:, :],
                    )

                def section(p):
                    units = [lambda p=p: u_pair_start(p)]
                    units.append(lambda p=p: u_scores(p, 0))
                    for kb in range(1, kmax):
                        units.append(lambda p=p, kb=kb: u_scores(p, kb))
                        units.append(lambda p=p, kb=kb: u_av(p, kb - 1))
                    units.append(lambda p=p: u_av(p, kmax - 1))
                    units.append(lambda p=p: u_norm(p))
                    return units

                sections_ab = section(0) + section(1)
                sections_cd = section(2) + section(3)
                proj_units = [
                    (lambda tb=tb, cc=cc: u_proj(tb, cc))
                    for tb in range(4)
                    for cc in range(2)
                ]
                return sections_ab, sections_cd, proj_units

            # ---------- emission schedule ----------
            # chunk0 first; windows qc=0..3 with fillers weighted by each
            # window's ACT-vs-PE deficit. chunk3's qk for p=2,3 slide into
            # window 3 (they are only needed by its p=2,3 sections).
            pre0, qk0 = qkv_chunk_units(0)
            for u in pre0 + qk0:
                u()

            pre1, qk1 = qkv_chunk_units(1)
            pre2, qk2 = qkv_chunk_units(2)
            pre3, qk3 = qkv_chunk_units(3)

            ab0, cd0, proj0 = att_chunk_units(0)
            for u in _interleave(ab0 + cd0, pre1 + qk1):
                u()

            ab1, cd1, proj1 = att_chunk_units(1)
            fill1 = pre2 + qk2 + [u_load_wp] + proj0[0:4]
            for u in _interleave(ab1 + cd1, fill1):
                u()

            ab2, cd2, proj2 = att_chunk_units(2)
            fill2 = pre3 + qk3[0:2] + proj0[4:8] + proj1[0:2]
            for u in _interleave(ab2 + cd2, fill2):
                u()

            ab3, cd3, proj3 = att_chunk_units(3)
            for u in _interleave(ab3, qk3[2:4] + proj1[2:8]):
                u()
            for u in _interleave(cd3, proj2):
                u()
            for u in proj3:
                u()

    nc.finalize()
    return nc


def _get_built():
    global _BUILT
    if _BUILT is None:
        _BUILT = _build()
    return _BUILT


def kernel(**inputs):
    global LAST_RESULT
    import ml_dtypes

    bfloat16 = ml_dtypes.bfloat16
    x = np.asarray(inputs["x"], dtype=np.float32)
    w_qkv = np.asarray(inputs["w_qkv"], dtype=np.float32)
    b_qkv = np.ascontiguousarray(np.asarray(inputs["b_qkv"], dtype=np.float32))
    w_proj = np.asarray(inputs["w_proj"], dtype=np.float32)
    b_proj = np.ascontiguousarray(np.asarray(inputs["b_proj"], dtype=np.float32))

    nc = _get_built()
    in_maps = []
    for c in range(N_CORES):
        b, hh = c // 2, c % 2
        s = 512 * hh
        in_maps.append(
            {
                "xbT": np.ascontiguousarray(x[b].T.astype(bfloat16)),
                "wq": np.ascontiguousarray(
                    w_qkv[:, s : s + 512].astype(bfloat16)
                ),
                "wk": np.ascontiguousarray(
                    w_qkv[:, 1024 + s : 1024 + s + 512].astype(bfloat16)
                ),
                "wv": np.ascontiguousarray(
                    w_qkv[:, 2048 + s : 2048 + s + 512].astype(bfloat16)
                ),
                "bq": np.ascontiguousarray(b_qkv[s : s + 512]),
                "bk": np.ascontiguousarray(b_qkv[1024 + s : 1024 + s + 512]),
                "bv": np.ascontiguousarray(b_qkv[2048 + s : 2048 + s + 512]),
                "wp": np.ascontiguousarray(
                    w_proj[s : s + 512, :].astype(bfloat16)
                ),
            }
        )

    trace = bool(int(os.environ.get("KERNEL_TRACE", "0")))
    res = run_bass_kernel_spmd(
        nc, in_maps, core_ids=list(range(N_CORES)), trace=trace
    )
    LAST_RESULT = res
    out = np.empty((B, T, C), dtype=np.float32)
    for b in range(B):
        out[b] = (
            res.results[2 * b]["y"] + res.results[2 * b + 1]["y"] + b_proj[None, :]
        )
    return out


# revision 8
# speedup vs baseline: 1.4373x; 1.0217x over previous
"""Causal self-attention (B=4, T=2048, C=1024, H=16, D=64) on 8 TRN2 cores.

Sharding: core c handles batch b = c//2 and head-half hh = c%2 (8 heads).
Each core computes qkv for its heads, attention, and a partial output
projection; the host sums the two partials per batch and adds b_proj.

Device kernel:
  - All matmul operands bf16 (f32 PSUM accumulate): halves LDWEIGHTS time
    so weight loads hide under the previous matmul's 512-col stream, and
    halves input DMA (host casts x/w to bf16).
  - x transposed on host (xT: [cin, t]) so qkv matmuls contract over cin.
  - q,k produced transposed per head-pair: qT/kT [128, T] bf16, partitions
    0:64 = head 2p, 64:128 = head 2p+1 (PE row-tiling runs the two K=64
    score matmuls concurrently).
  - scores as S^T [k, q] (k on partitions); att@v as out^T = v.T @ expS^T;
    v carries a ones column so the same accumulation produces the softmax
    denominator in output partition 64.
  - softmax normalization: DVE reciprocal_approx_fast + Pool
    partition_broadcast + one DVE multiply per head.
  - scores(kb+1) emitted before att@v(kb) (depth-2 software pipeline) so
    the ACT exp latency is covered by PE work in program order.
  - qkv chunk t4+1 and projection work are interleaved into attention
    windows weighted by each window's ACT-vs-PE deficit; chunk 3's q/k
    for head-pairs 2,3 slide into window 3 to feed its tail.
"""

import os

import numpy as np

import concourse.mybir as mybir
import concourse.tile as tile
from concourse import bacc
from concourse.bass_utils import run_bass_kernel_spmd

B, T, C = 4, 2048, 1024
H, D = 16, 64
HH = 512  # per-core head width: 8 heads * 64
N_CORES = 8

f32 = mybir.dt.float32
bf16 = mybir.dt.bfloat16
EXP = mybir.ActivationFunctionType.Exp

_BUILT = None
LAST_RESULT = None  # BassKernelResults of the most recent run (for profiling)


def _interleave(a, b):
    """Merge unit lists: spread b evenly through a."""
    out = []
    na, nb = len(a), len(b)
    if na == 0:
        return list(b)
    bi = 0
    for i, u in enumerate(a):
        out.append(u)
        while bi < nb and (bi + 1) * na <= (i + 1) * nb:
            out.append(b[bi])
            bi += 1
    out.extend(b[bi:])
    return out


def _build():
    nc = bacc.Bacc("TRN2", target_bir_lowering=False, debug=False)

    x_d = nc.dram_tensor("xbT", [C, T], bf16, kind="ExternalInput")
    wq_d = nc.dram_tensor("wq", [C, HH], bf16, kind="ExternalInput")
    wk_d = nc.dram_tensor("wk", [C, HH], bf16, kind="ExternalInput")
    wv_d = nc.dram_tensor("wv", [C, HH], bf16, kind="ExternalInput")
    bq_d = nc.dram_tensor("bq", [HH], f32, kind="ExternalInput")
    bk_d = nc.dram_tensor("bk", [HH], f32, kind="ExternalInput")
    bv_d = nc.dram_tensor("bv", [HH], f32, kind="ExternalInput")
    wp_d = nc.dram_tensor("wp", [HH, C], bf16, kind="ExternalInput")
    y_d = nc.dram_tensor("y", [T, C], f32, kind="ExternalOutput")

    with tile.TileContext(nc) as tc:
        with (
            tc.tile_pool(name="persist", bufs=1) as P0,
            tc.tile_pool(name="psum", bufs=3, space="PSUM") as PS,
            tc.tile_pool(name="acc", bufs=1, space="PSUM") as PA,
            tc.tile_pool(name="wpool", bufs=1) as PW,
            tc.tile_pool(name="ph1", bufs=2) as P1,
            tc.tile_pool(name="ph2", bufs=2) as P2,
            tc.tile_pool(name="oTp", bufs=3) as P2o,
            tc.tile_pool(name="expp", bufs=4) as PEx,
        ):
            # Resident weights first so their DMA queues ahead of masks.
            wvt = PW.tile([128, 8, HH], bf16, tag="wv")
            nc.sync.dma_start(
                wvt[:, :, :], wv_d[:, :].rearrange("(s u) m -> u s m", u=128)
            )
            # chunk-0 x slices issue before the wq/wk DMAs so the first
            # v matmuls are not stuck behind 4MB of q/k weights.
            xTc0 = P1.tile([128, 8, 512], bf16, tag="xT", name="xTc")
            for tbl in range(4):
                nc.sync.dma_start(
                    xTc0[:, :, tbl * 128 : (tbl + 1) * 128],
                    x_d[:, :].rearrange("(s u) t -> u s t", u=128)[
                        :, :, tbl * 128 : (tbl + 1) * 128
                    ],
                )
            wqt, wkt = [], []
            for p in range(4):
                wq_t = PW.tile([128, 8, 128], bf16, tag=f"wq{p}", name=f"wq{p}")
                nc.sync.dma_start(
                    wq_t[:, :, :],
                    wq_d[:, 128 * p : 128 * (p + 1)].rearrange(
                        "(s u) m -> u s m", u=128
                    ),
                )
                wqt.append(wq_t)
                wk_t = PW.tile([128, 8, 128], bf16, tag=f"wk{p}", name=f"wk{p}")
                nc.sync.dma_start(
                    wk_t[:, :, :],
                    wk_d[:, 128 * p : 128 * (p + 1)].rearrange(
                        "(s u) m -> u s m", u=128
                    ),
                )
                wkt.append(wk_t)

            # Multiplicative causal masks for the 4 diagonal-crossing
            # positions: keep S^T[k, q] iff q - k - 128*d >= 0.
            masks = []
            for d in range(4):
                m = P0.tile([128, 1024], bf16, tag=f"mask{d}", name=f"mask{d}")
                nc.gpsimd.memset(m[:, :], 1.0)
                mr = m[:, :].rearrange("p (h q) -> p h q", h=2)
                nc.gpsimd.affine_select(
                    out=mr,
                    in_=mr,
                    compare_op=mybir.AluOpType.is_ge,
                    fill=0.0,
                    base=-128 * d,
                    pattern=[[0, 2], [1, 512]],
                    channel_multiplier=-1,
                )
                masks.append(m)

            # ones_row: row 0 = 1.0, rest 0 (bias injection via extra
            # contraction block in the v matmul)
            ones_stage = P0.tile([128, 128], f32, tag="ones_stage")
            nc.gpsimd.memset(ones_stage[:, :], 0.0)
            nc.gpsimd.memset(ones_stage[0:1, :], 1.0)
            ones_row = P0.tile([128, 128], bf16, tag="ones_row")
            nc.vector.tensor_copy(ones_row[:, :], ones_stage[:, :])

            bqk_sb = P0.tile([128, 8], f32, tag="bqk")
            for p in range(4):
                nc.sync.dma_start(
                    bqk_sb[:, p : p + 1], bq_d[128 * p : 128 * (p + 1), None]
                )
                nc.sync.dma_start(
                    bqk_sb[:, 4 + p : 5 + p], bk_d[128 * p : 128 * (p + 1), None]
                )
            bv_stage = P0.tile([128, 512], f32, tag="bv_stage")
            nc.gpsimd.memset(bv_stage[:, :], 0.0)
            nc.sync.dma_start(bv_stage[0:1, :], bv_d[None, :])
            bv_row = P0.tile([128, 512], bf16, tag="bv_row")
            nc.vector.tensor_copy(bv_row[:, :], bv_stage[:, :])

            qT = [
                P0.tile([128, T], bf16, tag=f"qT{p}", name=f"qT{p}")
                for p in range(4)
            ]
            kT = [
                P0.tile([128, T], bf16, tag=f"kT{p}", name=f"kT{p}")
                for p in range(4)
            ]
            # v with a ones column per head: [t, kb, head, 65]; column 64
            # is 1.0 so att@v also accumulates the softmax denominator.
            v_sb = P0.tile([128, 16, 8, 65], bf16, tag="v")
            nc.gpsimd.memset(v_sb[:, :, :, 64:65], 1.0)

            # wp loaded lazily (first needed ~100us in, after window 0)
            wp_sb = P0.tile([128, 4, C], bf16, tag="wp")

            def u_load_wp():
                nc.sync.dma_start(
                    wp_sb[:, :, :],
                    wp_d[:, :].rearrange("(p u) c -> u p c", u=128),
                )

            # ---------- work-unit builders ----------

            def qkv_chunk_units(t4, xTc=None):
                """qkv for tokens [t4*512, (t4+1)*512): transposes, v, qT/kT.

                Returns (pre, qk) unit lists; qk[p] may be deferred into a
                later window than pre (loads + v), but every unit of pre
                must be emitted before any unit of qk.
                """
                loads, vs, qk = [], [], []
                cell = {} if xTc is None else {"xTc": xTc}

                def u_load(tbl, t4=t4, cell=cell):
                    if "xTc" not in cell:
                        cell["xTc"] = P1.tile(
                            [128, 8, 512], bf16, tag="xT", name="xTc"
                        )
                    xTc = cell["xTc"]
                    tb = 4 * t4 + tbl
                    nc.sync.dma_start(
                        xTc[:, :, tbl * 128 : (tbl + 1) * 128],
                        x_d[:, :].rearrange("(s u) t -> u s t", u=128)[
                            :, :, tb * 128 : (tb + 1) * 128
                        ],
                    )

                def u_v(tbl, t4=t4, cell=cell):
                    xTc = cell["xTc"]
                    tb = 4 * t4 + tbl
                    psv = PS.tile([128, 1024], f32, tag="s", name="psv")
                    for s in range(9):
                        lhsT = (
                            xTc[:, s, tbl * 128 : (tbl + 1) * 128]
                            if s < 8
                            else ones_row[:, :]
                        )
                        rhs = wvt[:, s, :] if s < 8 else bv_row[:, :]
                        nc.tensor.matmul(
                            psv[:, 0:512],
                            lhsT,
                            rhs,
                            start=(s == 0),
                            stop=(s == 8),
                        )
                    nc.vector.tensor_copy(
                        v_sb[:, tb, :, 0:64],
                        psv[:, 0:512].rearrange("p (h d) -> p h d", h=8),
                    )

                def u_qk(p, t4=t4, cell=cell):
                    xTc = cell["xTc"]
                    ps = PS.tile([128, 1024], f32, tag="s", name="psqk")
                    for s in range(8):
                        rhs = xTc[:, s, :]
                        nc.tensor.matmul(
                            ps[:, 0:512],
                            wqt[p][:, s, :],
                            rhs,
                            start=(s == 0),
                            stop=(s == 7),
                        )
                        nc.tensor.matmul(
                            ps[:, 512:1024],
                            wkt[p][:, s, :],
                            rhs,
                            start=(s == 0),
                            stop=(s == 7),
                        )
                    nc.vector.tensor_scalar_add(
                        qT[p][:, t4 * 512 : (t4 + 1) * 512],
                        ps[:, 0:512],
                        bqk_sb[:, p : p + 1],
                    )
                    nc.vector.tensor_scalar_add(
                        kT[p][:, t4 * 512 : (t4 + 1) * 512],
                        ps[:, 512:1024],
                        bqk_sb[:, 4 + p : 5 + p],
                    )

                for tbl in range(4):
                    loads.append(lambda tbl=tbl: u_load(tbl))
                    vs.append(lambda tbl=tbl: u_v(tbl))
                for p in range(4):
                    qk.append(lambda p=p: u_qk(p))
                return loads, vs, qk

            def att_chunk_units(qc):
                """Attention + projection for queries [qc*512, (qc+1)*512).

                Returns (sections_ab, sections_cd, proj_units): head-pair
                sections p=0,1 and p=2,3 as separate unit lists (so fillers
                that must precede p>=2 can interleave into the first half).
                Each u_scores(kb) is emitted one step ahead of u_av(kb-1):
                the ACT exp of kb-1 overlaps scores kb + fillers on PE.
                """
                kmax = 4 * qc + 4
                cell = {}

                def u_pair_start(p, cell=cell):
                    cell["oA"] = PA.tile([128, 512], f32, tag="poA", name="poA")
                    cell["oB"] = PA.tile([128, 512], f32, tag="poB", name="poB")

                def u_scores(p, kb, qc=qc, cell=cell):
                    # Diagonal-crossing blocks only have valid scores for
                    # q >= off = 128*dg: restrict matmul/exp/mask to that
                    # q-range (cols below off stay stale and unread).
                    dg = kb - 4 * qc
                    off = 128 * dg if dg > 0 else 0
                    ps_s = PS.tile([128, 1024], f32, tag="s", name="ps_s")
                    ksl = slice(kb * 128, (kb + 1) * 128)
                    qsl = slice(qc * 512 + off, (qc + 1) * 512)
                    nc.tensor.matmul(
                        ps_s[:, off:512],
                        kT[p][0:64, ksl],
                        qT[p][0:64, qsl],
                        start=True,
                        stop=True,
                    )
                    nc.tensor.matmul(
                        ps_s[:, 512 + off : 1024],
                        kT[p][64:128, ksl],
                        qT[p][64:128, qsl],
                        start=True,
                        stop=True,
                    )
                    e2 = PEx.tile([128, 1024], bf16, tag="e", name="e2")
                    if off:
                        e2r = e2[:, :].rearrange("p (h q) -> p h q", h=2)[
                            :, :, off:512
                        ]
                        psr = ps_s[:, :].rearrange("p (h q) -> p h q", h=2)[
                            :, :, off:512
                        ]
                        mr = masks[dg][:, :].rearrange(
                            "p (h q) -> p h q", h=2
                        )[:, :, off:512]
                        nc.scalar.activation(e2r, psr, EXP, scale=0.125)
                        nc.vector.tensor_mul(e2r, e2r, mr)
                    else:
                        nc.scalar.activation(
                            e2[:, :], ps_s[:, :], EXP, scale=0.125
                        )
                        if dg == 0:
                            nc.vector.tensor_mul(
                                e2[:, :], e2[:, :], masks[0][:, :]
                            )
                    cell[("e", kb)] = (e2, off)

                def u_av(p, kb, cell=cell, kmax=kmax):
                    e2, off = cell.pop(("e", kb))
                    first, last = kb == 0, kb == kmax - 1
                    nc.tensor.matmul(
                        cell["oA"][0:65, off:512],
                        v_sb[:, kb, 2 * p, :],
                        e2[:, off:512],
                        start=first,
                        stop=last,
                    )
                    nc.tensor.matmul(
                        cell["oB"][0:65, off:512],
                        v_sb[:, kb, 2 * p + 1, :],
                        e2[:, 512 + off : 1024],
                        start=first,
                        stop=last,
                    )

                def u_norm(p, cell=cell):
                    if "oT" not in cell:
                        cell["oT"] = P2o.tile(
                            [128, 4, 512], bf16, tag="oT", name="oT"
                        )
                    oT = cell["oT"]
                    # recip_approx_fast misreads partition-offset PSUM APs:
                    # stage both denominator rows to partition 0 first.
                    dst = P2.tile([1, 1024], f32, tag="rst", name="rst")
                    nc.vector.tensor_copy(dst[:, 0:512], cell["oA"][64:65, :])
                    nc.vector.tensor_copy(dst[:, 512:1024], cell["oB"][64:65, :])
                    rc = P2.tile([1, 1024], f32, tag="rc", name="rc")
                    nc.vector.reciprocal_approx_fast(out=rc[:, :], in_=dst[:, :])
                    bcA = P2.tile([64, 512], f32, tag="bcA", name="bcA")
                    bcB = P2.tile([64, 512], f32, tag="bcB", name="bcB")
                    nc.gpsimd.partition_broadcast(bcA[:, :], rc[:, 0:512])
                    nc.gpsimd.partition_broadcast(bcB[:, :], rc[:, 512:1024])
                    nc.vector.tensor_mul(
                        oT[0:64, p, :], cell["oA"][0:64, :], bcA[:, :]
                    )
                    nc.vector.tensor_mul(
                        oT[64:128, p, :], cell["oB"][0:64, :], bcB[:, :]
                    )

                def u_proj(tb, cc, qc=qc, cell=cell):
                    oT = cell["oT"]
                    psy = PS.tile([128, 1024], f32, tag="s", name="psy")
                    for p in range(4):
                        nc.tensor.matmul(
                            psy[:, 0:512],
                            oT[:, p, tb * 128 : (tb + 1) * 128],
                            wp_sb[:, p, cc * 512 : (cc + 1) * 512],
                            start=(p == 0),
                            stop=(p == 3),
                        )
                    yst = P2.tile([128, 512], f32, tag="yst", name="yst")
                    nc.vector.tensor_copy(yst[:, :], psy[:, 0:512])
                    r0 = qc * 512 + tb * 128
                    nc.sync.dma_start(
                        y_d[r0 : r0 + 128, cc * 512 : (cc + 1) * 512],
                        yst[:, :],
                    )

                def section(p):
                    units = [lambda p=p: u_pair_start(p)]
                    units.append(lambda p=p: u_scores(p, 0))
                    for kb in range(1, kmax):
                        units.append(lambda p=p, kb=kb: u_scores(p, kb))
                        units.append(lambda p=p, kb=kb: u_av(p, kb - 1))
                    units.append(lambda p=p: u_av(p, kmax - 1))
                    units.append(lambda p=p: u_norm(p))
                    return units

                sections_ab = section(0) + section(1)
                sections_cd = section(2) + section(3)
                proj_units = [
                    (lambda tb=tb, cc=cc: u_proj(tb, cc))
                    for tb in range(4)
                    for cc in range(2)
                ]
                return sections_ab, sections_cd, proj_units

            # ---------- emission schedule ----------
            # chunk0 first (x already prefetched into xTc0); windows
            # qc=0..3 with fillers weighted by each window's ACT-vs-PE
            # deficit. chunk3's qk for p=2,3 slide into window 3 (they
            # are only needed by its p=2,3 sections).
            _l0, v0_units, qk0 = qkv_chunk_units(0, xTc=xTc0)
            for u in v0_units + qk0:
                u()

            l1, v1, qk1 = qkv_chunk_units(1)
            l2, v2, qk2 = qkv_chunk_units(2)
            l3, v3, qk3 = qkv_chunk_units(3)

            ab0, cd0, proj0 = att_chunk_units(0)
            for u in _interleave(ab0 + cd0, l1 + v1 + qk1):
                u()

            ab1, cd1, proj1 = att_chunk_units(1)
            fill1 = l2 + v2 + qk2 + [u_load_wp] + proj0[0:4]
            for u in _interleave(ab1 + cd1, fill1):
                u()

            ab2, cd2, proj2 = att_chunk_units(2)
            fill2 = l3 + v3 + qk3[0:2] + proj0[4:8] + proj1[0:2]
            for u in _interleave(ab2 + cd2, fill2):
                u()

            ab3, cd3, proj3 = att_chunk_units(3)
            for u in _interleave(ab3, qk3[2:4] + proj1[2:8]):
                u()
            for u in _interleave(cd3, proj2):
                u()
            for u in proj3:
                u()

    nc.finalize()
    return nc


def _get_built():
    global _BUILT
    if _BUILT is None:
        _BUILT = _build()
    return _BUILT


def kernel(**inputs):
    global LAST_RESULT
    import ml_dtypes

    bfloat16 = ml_dtypes.bfloat16
    x = np.asarray(inputs["x"], dtype=np.float32)
    w_qkv = np.asarray(inputs["w_qkv"], dtype=np.float32)
    b_qkv = np.ascontiguousarray(np.asarray(inputs["b_qkv"], dtype=np.float32))
    w_proj = np.asarray(inputs["w_proj"], dtype=np.float32)
    b_proj = np.ascontiguousarray(np.asarray(inputs["b_proj"], dtype=np.float32))

    nc = _get_built()
    in_maps = []
    for c in range(N_CORES):
        b, hh = c // 2, c % 2
        s = 512 * hh
        in_maps.append(
            {
                "xbT": np.ascontiguousarray(x[b].T.astype(bfloat16)),
                "wq": np.ascontiguousarray(
                    w_qkv[:, s : s + 512].astype(bfloat16)
                ),
                "wk": np.ascontiguousarray(
                    w_qkv[:, 1024 + s : 1024 + s + 512].astype(bfloat16)
                ),
                "wv": np.ascontiguousarray(
                    w_qkv[:, 2048 + s : 2048 + s + 512].astype(bfloat16)
                ),
                "bq": np.ascontiguousarray(b_qkv[s : s + 512]),
                "bk": np.ascontiguousarray(b_qkv[1024 + s : 1024 + s + 512]),
                "bv": np.ascontiguousarray(b_qkv[2048 + s : 2048 + s + 512]),
                "wp": np.ascontiguousarray(
                    w_proj[s : s + 512, :].astype(bfloat16)
                ),
            }
        )

    trace = bool(int(os.environ.get("KERNEL_TRACE", "0")))
    res = run_bass_kernel_spmd(
        nc, in_maps, core_ids=list(range(N_CORES)), trace=trace
    )
    LAST_RESULT = res
    out = np.empty((B, T, C), dtype=np.float32)
    for b in range(B):
        out[b] = (
            res.results[2 * b]["y"] + res.results[2 * b + 1]["y"] + b_proj[None, :]
        )
    return out
